# revision 38
# baseline (speedup 1.0000x reference)
"""Trainium2 Bass kernel for nn_AttentionBlock (B=2, S=4096, HID=256, 8 heads).

Sharding: 8 cores = 2 batches x 4 query-chunks of 1024 queries.
Each core redundantly computes full K/V projections for its batch (over the
mask-compacted key set), then attention for its 1024 queries over all 8
heads, then the output projection. Host gathers by concatenation.

Key structure (v2):
- Mask compaction: surviving key indices gathered via indirect DMA from a
  host-concatenated [K|V] tensor with appended zero rows; padding slots
  point at the zero rows, and the ones-column of the augmented V carries the
  mask, so masking is exact with no exp bias anywhere.
- Scores computed transposed (sT[k,q]) via 4-way row-packed K=32 fp16
  matmuls, 512 queries per matmul.
- Softmax exp split across engines per head-pair: half the head-pairs use
  the Scalar engine's LUT exp; the other half use a two-sample Schraudolph
  bit-trick exp on the Vector engine (t = A*x + B rounded to int16,
  bitcast to fp16 = 2^(t/1024) approx; summing the B and B+512 samples
  cancels the sawtooth to ~0.5% ripple; the per-softmax constant factor
  cancels in normalization). The extra sample is accumulated by the PE via
  a second PV matmul wave into the same PSUM accumulators.
- Augmented V tiles [128, 512]: per head 64 cols = [32 v | mask | 31 zeros],
  so M=64 PV matmuls fully cover the PSUM rows (no garbage rows) and the
  denominators accumulate alongside.
- Normalization: denominator rows DMA-packed into one [8,512] tile per qc,
  reciprocal via int32 magic-constant + 2 Newton iterations on DVE (scaled
  by 2048 to keep fp16 r values normal), scattered to partitions 32/33 of a
  small tile, broadcast to all 128 partitions with one K=2 matmul against a
  1/2048-valued selector, then one full-tile multiply produces the fp16
  normalized weights for the output projection.
- Output projection: fused K=128 matmuls against zero-padded Wo rows, bias
  (incl. folded bv@Wo) via a K=1 ones matmul.
"""

import numpy as np

import concourse.bacc as bacc
import concourse.bass as bass
from concourse import mybir
from concourse.tile import TileContext
from concourse.masks import make_identity
from concourse.bass_utils import run_bass_kernel_spmd

F32 = mybir.dt.float32
F16 = mybir.dt.float16
I16 = mybir.dt.int16
I32 = mybir.dt.int32
AF = mybir.ActivationFunctionType
ALU = mybir.AluOpType

HID = 256
HEADS = 8
DH = 32
SK = 4096
SKP = 4104  # K/V rows incl. zero-pad rows
SQ = 1024   # queries per core
SCALE = 1.0 / np.sqrt(32.0)
A16 = 1024.0 / np.log(2.0)          # Schraudolph slope (fp16 format)
ASC = A16 * SCALE                   # folded score scale
B16 = 15360.0                       # Schraudolph offset (15*1024)
MAGIC = 0x7EF311C3                  # fp32 reciprocal magic
RS = 2048.0                         # reciprocal output scaling (keeps fp16 normal)

_CACHE = {}

# exp-engine map: (qc, g, jj) -> True if DVE (Schraudolph), False if ACT
def _use_dve(qc, g, jj):
    return jj == 1


def _build_nc(nkc):
    """nkc = number of 128-key tiles after mask compaction (multiple of 4)."""
    skc = nkc * 128
    nc = bacc.Bacc("TRN2", target_bir_lowering=False, debug=False, num_devices=8)

    q_d = nc.dram_tensor("q_in", [SQ, HID], F16, kind="ExternalInput").ap()
    kvh_d = nc.dram_tensor("kv_in", [SKP, 2 * HID], F16, kind="ExternalInput").ap()
    vg_d = nc.dram_tensor("vginit", [128, nkc * 512], F16, kind="ExternalInput").ap()
    i_d = nc.dram_tensor("idx_in", [128, nkc], I32, kind="ExternalInput").ap()
    wq_d = nc.dram_tensor("wq", [HID, HID], F32, kind="ExternalInput").ap()
    wk_d = nc.dram_tensor("wk", [HID, HID], F32, kind="ExternalInput").ap()
    wv_d = nc.dram_tensor("wv", [HID, HID], F32, kind="ExternalInput").ap()
    wo_d = nc.dram_tensor("wo_arr", [128, 1024], F32, kind="ExternalInput").ap()
    bq_d = nc.dram_tensor("bq2", [128, 2], F32, kind="ExternalInput").ap()
    bk_d = nc.dram_tensor("bk2", [128, 2], F32, kind="ExternalInput").ap()
    bo_d = nc.dram_tensor("bo2", [1, HID], F32, kind="ExternalInput").ap()
    sel_d = nc.dram_tensor("sel2", [2, 128], F16, kind="ExternalInput").ap()
    out_d = nc.dram_tensor("out", [SQ, HID], F32, kind="ExternalOutput").ap()

    from contextlib import ExitStack

    with TileContext(nc) as tc, ExitStack() as top:
        const = top.enter_context(tc.tile_pool(name="const", bufs=1))
        persist = top.enter_context(tc.tile_pool(name="persist", bufs=1))
        io_pool = top.enter_context(tc.tile_pool(name="io", bufs=16))
        xt_pool = top.enter_context(tc.tile_pool(name="xt", bufs=3))
        vt_pool = top.enter_context(tc.tile_pool(name="vt", bufs=3))
        pt_pool = top.enter_context(tc.tile_pool(name="pt", bufs=6))
        wc_pool = top.enter_context(tc.tile_pool(name="wc", bufs=8))
        dn_pool = top.enter_context(tc.tile_pool(name="dn", bufs=18))
        rs_pool = top.enter_context(tc.tile_pool(name="rs", bufs=8))
        osb_pool = top.enter_context(tc.tile_pool(name="osb", bufs=4))

        # phase-A PSUM pools are scoped: closed before attention so their
        # banks go to a third st buffer (deeper score->exp pipeline)
        phaseA_stack = ExitStack()
        tpsum = phaseA_stack.enter_context(
            tc.tile_pool(name="tpsum", bufs=2, space="PSUM"))
        ppsum = phaseA_stack.enter_context(
            tc.tile_pool(name="ppsum", bufs=1, space="PSUM"))

        # ---------------- early IO: idx, gathers, q loads ----------------
        idx_sb = const.tile([128, nkc], I32, name="idx_sb")
        nc.sync.dma_start(idx_sb, i_d)

        xkv_tiles = {}

        def gather_kv(s):
            xkv = io_pool.tile([128, 512], F16, tag="xkv", name="xkv")
            nc.gpsimd.indirect_dma_start(
                out=xkv, out_offset=None, in_=kvh_d,
                in_offset=bass.IndirectOffsetOnAxis(
                    ap=idx_sb[:, s:s + 1], axis=0))
            xkv_tiles[s] = xkv

        for s in range(min(8, nkc)):
            gather_kv(s)

        xq_all = []
        for s in range(8):
            xq = io_pool.tile([128, 256], F16, tag="xq", name="xq")
            nc.sync.dma_start(xq, q_d[s * 128:(s + 1) * 128, :])
            xq_all.append(xq)

        # ---------------- constants ----------------
        wq_hf = []
        wk_hf = []
        wv_hf = []
        for t in range(2):
            for nm, d_ap, lst in (("wq", wq_d, wq_hf), ("wk", wk_d, wk_hf),
                                  ("wv", wv_d, wv_hf)):
                wf = const.tile([128, 256], F32, name=f"{nm}_f{t}")
                nc.sync.dma_start(wf, d_ap[t * 128:(t + 1) * 128, :])
                wb = const.tile([128, 256], F16, name=f"{nm}_h{t}")
                nc.vector.tensor_copy(wb, wf)
                lst.append(wb)
        wo_f = const.tile([128, 1024], F32, name="wo_f")
        nc.scalar.dma_start(wo_f, wo_d)
        wo_hf = const.tile([128, 1024], F16, name="wo_hf")
        nc.vector.tensor_copy(wo_hf, wo_f)
        bq_sb = const.tile([128, 2], F32, name="bq_sb")
        nc.sync.dma_start(bq_sb, bq_d)
        bk_sb = const.tile([128, 2], F32, name="bk_sb")
        nc.sync.dma_start(bk_sb, bk_d)
        bo_f = const.tile([1, HID], F32, name="bo_f")
        nc.scalar.dma_start(bo_f, bo_d)
        bo_hf = const.tile([1, HID], F16, name="bo_hf")
        nc.vector.tensor_copy(bo_hf, bo_f)

        identity = const.tile([128, 128], F32, name="identity")
        make_identity(nc, identity)
        ident_h = const.tile([128, 128], F16, name="ident_h")
        nc.vector.tensor_copy(ident_h, identity)
        ones_hf = const.tile([1, 128], F16, name="ones_hf")
        nc.vector.memset(ones_hf, 1.0)
        # K=2 broadcast selector at partitions 32/33 (value 1/2048 = 2^-11)
        sel2 = const.tile([128, 128], F16, name="sel2")
        nc.sync.dma_start(sel2[32:34, :], sel_d)

        # ---------------- persistent buffers ----------------
        qT_sb = [persist.tile([128, SQ], F16, name=f"qT_sb{g}") for g in range(2)]
        kT_ch = [[persist.tile([128, 512], F16, name=f"kT{g}_{c}")
                  for c in range(skc // 512)] for g in range(2)]
        # augmented V: per head 64 cols = [32 v | mask | 31 zeros]
        vaug = [persist.tile([128, 512], F16, name=f"vaug{s}")
                for s in range(nkc)]
        wtn_all = [persist.tile([128, 512], F16, name=f"wtn{i}")
                   for i in range(4)]
        # vaug init from host (mask col + zeros; v cols overwritten by proj)
        for s in range(nkc):
            nc.scalar.dma_start(vaug[s], vg_d[:, s * 512:(s + 1) * 512])

        # ---------------- phase A helpers ----------------
        def transpose_half(xins, col0):
            """4 tiles -> fp16 [128, 512] chunk of rows col0..col0+128."""
            tp = tpsum.tile([128, 512], F16, tag="tp", name="tp")
            for j in range(4):
                nc.tensor.transpose(
                    tp[:, j * 128:(j + 1) * 128],
                    xins[j][:, col0:col0 + 128], ident_h)
            ch = xt_pool.tile([128, 512], F16, tag="xch", name="xch")
            nc.vector.tensor_copy(ch, tp)
            return ch

        def project_chunk(chunks, w_hf, b_sb, outs):
            for g in range(2):
                ps = ppsum.tile([128, 512], F32, tag="proj", name="ps")
                for t in range(2):
                    nc.tensor.matmul(
                        ps, w_hf[t][:, g * 128:(g + 1) * 128], chunks[t],
                        start=(t == 0), stop=(t == 1))
                nc.scalar.activation(outs[g], ps, AF.Identity,
                                     bias=b_sb[:, g:g + 1], scale=1.0)

        def value_tile(s, xkv):
            vtp = tpsum.tile([128, 256], F16, tag="tp", name="vtp")
            for t in range(2):
                nc.tensor.transpose(
                    vtp[:, t * 128:(t + 1) * 128],
                    xkv[:, 256 + t * 128:256 + (t + 1) * 128], ident_h)
            vT = vt_pool.tile([128, 256], F16, tag="vT", name="vT")
            nc.scalar.activation(vT, vtp, AF.Copy)
            vps = ppsum.tile([128, 256], F32, tag="proj", name="vps")
            for t in range(2):
                nc.tensor.matmul(
                    vps, vT[:, t * 128:(t + 1) * 128], wv_hf[t],
                    start=(t == 0), stop=(t == 1))
            dst = vaug[s].rearrange("p (h e) -> p h e", e=64)[:, :, 0:DH]
            src = vps.rearrange("p (h e) -> p h e", e=DH)
            nc.vector.tensor_copy(dst, src)

        # ---------------- phase A emission ----------------
        def emit_kv_chunk(cch):
            xins = [xkv_tiles[cch * 4 + j] for j in range(4)]
            chunks = [transpose_half(xins, t * 128) for t in range(2)]
            project_chunk(chunks, wk_hf, bk_sb,
                          [kT_ch[g][cch] for g in range(2)])
            for j in range(4):
                value_tile(cch * 4 + j, xins[j])

        nch = skc // 512
        for s in range(8, nkc):
            gather_kv(s)
        for sg in range(SQ // 512):
            xq = xq_all[sg * 4:sg * 4 + 4]
            chunks = [transpose_half(xq, t * 128) for t in range(2)]
            project_chunk(chunks, wq_hf, bq_sb,
                          [qT_sb[g][:, sg * 512:(sg + 1) * 512]
                           for g in range(2)])
        for cch in range(nch):
            emit_kv_chunk(cch)

        # close phase-A PSUM pools, open attention pools over their banks
        phaseA_stack.close()
        st_pool = top.enter_context(
            tc.tile_pool(name="stp", bufs=3, space="PSUM"))
        wt_pool = top.enter_context(
            tc.tile_pool(name="wtp", bufs=2, space="PSUM"))

        # ---------------- phase B: attention ----------------
        def make_tail(qc, wcops, dpack):
            def tail():
                # reciprocal: magic + 2 Newton iterations, output 2048/d fp16
                r0i = dn_pool.tile([8, 512], I32, tag="dp", name="r0i")
                nc.vector.tensor_scalar(r0i, dpack.bitcast(I32), -1, MAGIC,
                                        op0=ALU.mult, op1=ALU.add)
                r0 = r0i.bitcast(F32)
                t1 = dn_pool.tile([8, 512], F32, tag="dp", name="t1")
                nc.vector.tensor_tensor(t1, dpack, r0, op=ALU.mult)
                t1b = dn_pool.tile([8, 512], F32, tag="dp", name="t1b")
                nc.vector.tensor_scalar(t1b, t1, -1.0, 2.0,
                                        op0=ALU.mult, op1=ALU.add)
                r1 = dn_pool.tile([8, 512], F32, tag="dp", name="r1")
                nc.vector.tensor_tensor(r1, r0, t1b, op=ALU.mult)
                t2 = dn_pool.tile([8, 512], F32, tag="dp", name="t2")
                nc.vector.tensor_tensor(t2, dpack, r1, op=ALU.mult)
                t2b = dn_pool.tile([8, 512], F32, tag="dp", name="t2b")
                nc.vector.tensor_scalar(t2b, t2, -RS, 2.0 * RS,
                                        op0=ALU.mult, op1=ALU.add)
                r2h = dn_pool.tile([8, 512], F16, tag="dp", name="r2h")
                nc.vector.tensor_tensor(r2h, r1, t2b, op=ALU.mult)

                # broadcast + normalize
                for g in range(2):
                    for jj in range(2):
                        p = 2 * g + jj
                        rsp = rs_pool.tile([34, 512], F16, tag="rsp",
                                           name="rsp")
                        r = 4 * g + 2 * jj
                        nc.sync.dma_start(rsp[32:33, :], r2h[r:r + 1, :])
                        nc.sync.dma_start(rsp[33:34, :], r2h[r + 1:r + 2, :])
                        bc = wt_pool.tile([128, 512], F32, tag="wt", name="bc")
                        nc.tensor.matmul(bc, sel2[32:34, :], rsp[32:34, :],
                                         start=True, stop=True,
                                         tile_position=(32, 0))
                        nc.vector.tensor_tensor(wtn_all[p], wcops[p], bc,
                                                op=ALU.mult)

                # output projection
                for m in range(4):
                    opsb = wt_pool.tile([128, 512], F32, tag="wt", name="ops")
                    ops = opsb[:, 0:256]
                    for p in range(4):
                        nc.tensor.matmul(
                            ops, wtn_all[p][:, m * 128:(m + 1) * 128],
                            wo_hf[:, p * 256:(p + 1) * 256],
                            start=(p == 0), stop=False,
                            skip_group_check=True)
                    nc.tensor.matmul(ops, ones_hf[0:1, :], bo_hf,
                                     start=False, stop=True,
                                     skip_group_check=True)
                    ob = osb_pool.tile([128, 256], F32, tag="ob", name="ob")
                    nc.scalar.activation(ob, ops, AF.Copy)
                    nc.sync.dma_start(
                        out_d[qc * 512 + m * 128:qc * 512 + (m + 1) * 128, :],
                        ob)
            return tail

        pending_tail = None
        for qc in range(SQ // 512):
            wcops = []
            dpack = dn_pool.tile([8, 512], F32, tag="dp", name="dpack")
            for g in range(2):
                wts = [wt_pool.tile([128, 512], F32, tag="wt", name=f"wt{jj}")
                       for jj in range(2)]
                for kt in range(nkc):
                    for jj in range(2):
                        st = st_pool.tile([128, 1024], F32, tag="st",
                                          name="st")
                        for j2 in range(2):
                            j = 2 * jj + j2
                            nc.tensor.matmul(
                                st[:, j2 * 512:(j2 + 1) * 512],
                                kT_ch[g][kt // 4][32 * j:32 * j + 32,
                                                  (kt % 4) * 128:
                                                  (kt % 4) * 128 + 128],
                                qT_sb[g][32 * j:32 * j + 32,
                                         qc * 512:(qc + 1) * 512],
                                start=True, stop=True,
                                tile_position=(32 * j, 0))
                        if _use_dve(qc, g, jj):
                            p1 = pt_pool.tile([128, 1024], I16, tag="pt",
                                              name="p1")
                            nc.vector.tensor_scalar(
                                p1, st, ASC, B16,
                                op0=ALU.mult, op1=ALU.add)
                            p2 = pt_pool.tile([128, 1024], I16, tag="pt",
                                              name="p2")
                            nc.gpsimd.tensor_scalar(
                                p2, p1, 1, 512,
                                op0=ALU.mult, op1=ALU.add)
                            streams = [p1.bitcast(F16), p2.bitcast(F16)]
                        else:
                            pt = pt_pool.tile([128, 1024], F16, tag="pt",
                                              name="pt")
                            nc.scalar.activation(pt, st, AF.Exp,
                                                 scale=SCALE)
                            streams = [pt]
                        nstr = len(streams)
                        for si, pstr in enumerate(streams):
                            for j2 in range(2):
                                h = 4 * g + 2 * jj + j2
                                nc.tensor.matmul(
                                    wts[jj][64 * j2:64 * j2 + 64, :],
                                    vaug[kt][:, 64 * h:64 * h + 64],
                                    pstr[:, j2 * 512:(j2 + 1) * 512],
                                    start=(kt == 0 and si == 0),
                                    stop=(kt == nkc - 1 and si == nstr - 1),
                                    tile_position=(0, 64 * j2),
                                    skip_group_check=True)



                # evict (frees PSUM), stash denominator rows
                for jj in range(2):
                    wcop = wc_pool.tile([128, 512], F32, tag="wcop",
                                        name="wcop")
                    nc.scalar.activation(wcop, wts[jj], AF.Copy)
                    r = 4 * g + 2 * jj
                    nc.sync.dma_start(dpack[r:r + 1, :], wcop[32:33, :])
                    nc.sync.dma_start(dpack[r + 1:r + 2, :], wcop[96:97, :])
                    wcops.append(wcop)

                # overlap previous qc's normalize+outproj under this qc
                if pending_tail is not None and g == 0:
                    pending_tail()
                    pending_tail = None

            pending_tail = make_tail(qc, wcops, dpack)
        pending_tail()

    nc.finalize()
    return nc


def _get_nc(nkc):
    key = ("nc", nkc)
    if key not in _CACHE:
        _CACHE[key] = _build_nc(nkc)
    return _CACHE[key]


def kernel(query, key, value, mask, Wq, bq, Wk, bk, Wv, bv, Wo, bo,
           _trace=False):
    query = np.asarray(query, np.float32)
    key = np.asarray(key, np.float32)
    value = np.asarray(value, np.float32)
    mask = np.asarray(mask, np.int32)
    Wq = np.ascontiguousarray(np.asarray(Wq, np.float32))
    Wk = np.ascontiguousarray(np.asarray(Wk, np.float32))
    Wv = np.ascontiguousarray(np.asarray(Wv, np.float32))
    Wo = np.ascontiguousarray(np.asarray(Wo, np.float32))
    bq = np.asarray(bq, np.float32)
    bk = np.asarray(bk, np.float32)
    bv = np.asarray(bv, np.float32)
    bo = np.asarray(bo, np.float32)

    # mask compaction: indices of surviving keys per batch, padded to a
    # multiple of 512 with pointers at the zero rows
    idxs = [np.nonzero(mask[b, 0])[0].astype(np.int32) for b in range(2)]
    nk_max = max(len(ix) for ix in idxs)
    nk_max = max(nk_max, 1)
    skc = ((nk_max + 511) // 512) * 512
    nkc = skc // 128

    nc = _get_nc(nkc)

    wo_arr = np.zeros((128, 4, 256), np.float32)
    for p in range(4):
        wo_arr[0:32, p] = Wo[64 * p:64 * p + 32]
        wo_arr[64:96, p] = Wo[64 * p + 32:64 * p + 64]
    wo_arr = np.ascontiguousarray(wo_arr.reshape(128, 1024))
    bq2 = np.ascontiguousarray(bq.reshape(2, 128).T)
    bk2 = np.ascontiguousarray(bk.reshape(2, 128).T)
    bo2 = np.ascontiguousarray((bv @ Wo + bo).reshape(1, 256))
    sel2 = np.zeros((2, 128), np.float16)
    sel2[0, 0:32] = 1.0 / RS
    sel2[1, 64:96] = 1.0 / RS

    # per-batch KV concat with zero pad rows (fp16 on host)
    kv_full = []
    for b in range(2):
        kv = np.zeros((SKP, 2 * HID), np.float16)
        kv[:SK, :HID] = key[b].astype(np.float16)
        kv[:SK, HID:] = value[b].astype(np.float16)
        kv_full.append(kv)

    in_maps = []
    for c in range(8):
        b, qi = divmod(c, 4)
        ix = idxs[b]
        nk = len(ix)
        ix_pad = np.concatenate(
            [ix, np.full(skc - nk, SK, np.int32)])
        mcomp = (np.arange(skc) < nk).astype(np.float16)
        ib = np.ascontiguousarray(ix_pad.reshape(nkc, 128).T)
        # vaug init pattern: per tile, per head: [32 zeros | mask | 31 zeros]
        mb = np.ascontiguousarray(mcomp.reshape(nkc, 128).T)  # [128, nkc]
        vgi = np.zeros((128, nkc, 8, 64), np.float16)
        vgi[:, :, :, 32] = mb[:, :, None]
        vgi = np.ascontiguousarray(vgi.reshape(128, nkc * 512))
        in_maps.append({
            "q_in": np.ascontiguousarray(
                query[b, qi * SQ:(qi + 1) * SQ].astype(np.float16)),
            "kv_in": kv_full[b],
            "vginit": vgi,
            "idx_in": ib,
            "wq": Wq, "wk": Wk, "wv": Wv, "wo_arr": wo_arr,
            "bq2": bq2, "bk2": bk2, "bo2": bo2, "sel2": sel2,
        })

    res = run_bass_kernel_spmd(nc, in_maps, core_ids=list(range(8)),
                               trace=_trace)
    if _trace:
        _CACHE["last_result"] = res

    out = np.empty((2, 4096, HID), np.float32)
    for c in range(8):
        b, qi = divmod(c, 4)
        out[b, qi * SQ:(qi + 1) * SQ] = res.results[c]["out"]
    return out


# revision 40
# speedup vs baseline: 1.0640x; 1.0640x over previous
"""Trainium2 Bass kernel for nn_AttentionBlock (B=2, S=4096, HID=256, 8 heads).

Sharding: 8 cores = 2 batches x 4 query-chunks of 1024 queries.
Each core redundantly computes full K/V projections for its batch (over the
mask-compacted key set), then attention for its 1024 queries over all 8
heads, then the output projection. Host gathers by concatenation.

Key structure (v2):
- Mask compaction: surviving key indices gathered via indirect DMA from a
  host-concatenated [K|V] tensor with appended zero rows; padding slots
  point at the zero rows, and the ones-column of the augmented V carries the
  mask, so masking is exact with no exp bias anywhere.
- Scores computed transposed (sT[k,q]) via 4-way row-packed K=32 fp16
  matmuls, 512 queries per matmul.
- Softmax exp split across engines per head-pair: half the head-pairs use
  the Scalar engine's LUT exp; the other half use a two-sample Schraudolph
  bit-trick exp on the Vector engine (t = A*x + B rounded to int16,
  bitcast to fp16 = 2^(t/1024) approx; summing the B and B+512 samples
  cancels the sawtooth to ~0.5% ripple; the per-softmax constant factor
  cancels in normalization). The extra sample is accumulated by the PE via
  a second PV matmul wave into the same PSUM accumulators.
- Augmented V tiles [128, 512]: per head 64 cols = [32 v | mask | 31 zeros],
  so M=64 PV matmuls fully cover the PSUM rows (no garbage rows) and the
  denominators accumulate alongside.
- Normalization: denominator rows DMA-packed into one [8,512] tile per qc,
  reciprocal via int32 magic-constant + 2 Newton iterations on DVE (scaled
  by 2048 to keep fp16 r values normal), scattered to partitions 32/33 of a
  small tile, broadcast to all 128 partitions with one K=2 matmul against a
  1/2048-valued selector, then one full-tile multiply produces the fp16
  normalized weights for the output projection.
- Output projection: fused K=128 matmuls against zero-padded Wo rows, bias
  (incl. folded bv@Wo) via a K=1 ones matmul.
"""

import numpy as np

import concourse.bacc as bacc
import concourse.bass as bass
from concourse import mybir
from concourse.tile import TileContext
from concourse.masks import make_identity
from concourse.bass_utils import run_bass_kernel_spmd

F32 = mybir.dt.float32
F16 = mybir.dt.float16
I16 = mybir.dt.int16
I32 = mybir.dt.int32
AF = mybir.ActivationFunctionType
ALU = mybir.AluOpType

HID = 256
HEADS = 8
DH = 32
SK = 4096
SKP = 4104  # K/V rows incl. zero-pad rows
SQ = 1024   # queries per core
SCALE = 1.0 / np.sqrt(32.0)
A16 = 1024.0 / np.log(2.0)          # Schraudolph slope (fp16 format)
ASC = A16 * SCALE                   # folded score scale
B16 = 15360.0                       # Schraudolph offset (15*1024)
MAGIC = 0x7EF311C3                  # fp32 reciprocal magic
RS = 2048.0                         # reciprocal output scaling (keeps fp16 normal)

_CACHE = {}

# exp-engine map: (qc, g, jj) -> True if DVE (Schraudolph), False if ACT
def _use_dve(qc, g, jj):
    return jj == 1


def _build_nc(nkc):
    """nkc = number of 128-key tiles after mask compaction (multiple of 4)."""
    skc = nkc * 128
    nc = bacc.Bacc("TRN2", target_bir_lowering=False, debug=False, num_devices=8)

    q_d = nc.dram_tensor("q_in", [SQ, HID], F16, kind="ExternalInput").ap()
    kvh_d = nc.dram_tensor("kv_in", [SKP, 2 * HID], F16, kind="ExternalInput").ap()
    vg_d = nc.dram_tensor("vginit", [128, nkc * 512], F16, kind="ExternalInput").ap()
    i_d = nc.dram_tensor("idx_in", [128, nkc], I32, kind="ExternalInput").ap()
    wq_d = nc.dram_tensor("wq", [HID, HID], F32, kind="ExternalInput").ap()
    wk_d = nc.dram_tensor("wk", [HID, HID], F32, kind="ExternalInput").ap()
    wv_d = nc.dram_tensor("wv", [HID, HID], F32, kind="ExternalInput").ap()
    wo_d = nc.dram_tensor("wo_arr", [128, 1024], F32, kind="ExternalInput").ap()
    bq_d = nc.dram_tensor("bq2", [128, 2], F32, kind="ExternalInput").ap()
    bk_d = nc.dram_tensor("bk2", [128, 2], F32, kind="ExternalInput").ap()
    bo_d = nc.dram_tensor("bo2", [1, HID], F32, kind="ExternalInput").ap()
    sel_d = nc.dram_tensor("sel2", [2, 128], F16, kind="ExternalInput").ap()
    out_d = nc.dram_tensor("out", [SQ, HID], F32, kind="ExternalOutput").ap()

    from contextlib import ExitStack

    with TileContext(nc) as tc, ExitStack() as top:
        const = top.enter_context(tc.tile_pool(name="const", bufs=1))
        persist = top.enter_context(tc.tile_pool(name="persist", bufs=1))
        io_pool = top.enter_context(tc.tile_pool(name="io", bufs=16))
        xt_pool = top.enter_context(tc.tile_pool(name="xt", bufs=3))
        vt_pool = top.enter_context(tc.tile_pool(name="vt", bufs=3))
        pt_pool = top.enter_context(tc.tile_pool(name="pt", bufs=6))
        wc_pool = top.enter_context(tc.tile_pool(name="wc", bufs=8))
        dn_pool = top.enter_context(tc.tile_pool(name="dn", bufs=18))
        rs_pool = top.enter_context(tc.tile_pool(name="rs", bufs=8))
        osb_pool = top.enter_context(tc.tile_pool(name="osb", bufs=4))

        # phase-A PSUM pools are scoped: closed before attention so their
        # banks go to a third st buffer (deeper score->exp pipeline)
        phaseA_stack = ExitStack()
        tpsum = phaseA_stack.enter_context(
            tc.tile_pool(name="tpsum", bufs=2, space="PSUM"))
        ppsum = phaseA_stack.enter_context(
            tc.tile_pool(name="ppsum", bufs=3, space="PSUM"))

        # ---------------- early IO: idx, gathers, q loads ----------------
        idx_sb = const.tile([128, nkc], I32, name="idx_sb")
        nc.sync.dma_start(idx_sb, i_d)

        xkv_tiles = {}

        def gather_kv(s):
            xkv = io_pool.tile([128, 512], F16, tag="xkv", name="xkv")
            nc.gpsimd.indirect_dma_start(
                out=xkv, out_offset=None, in_=kvh_d,
                in_offset=bass.IndirectOffsetOnAxis(
                    ap=idx_sb[:, s:s + 1], axis=0))
            xkv_tiles[s] = xkv

        for s in range(min(8, nkc)):
            gather_kv(s)

        xq_all = []
        for s in range(8):
            xq = io_pool.tile([128, 256], F16, tag="xq", name="xq")
            nc.sync.dma_start(xq, q_d[s * 128:(s + 1) * 128, :])
            xq_all.append(xq)

        # ---------------- constants ----------------
        wq_hf = []
        wk_hf = []
        wv_hf = []
        for t in range(2):
            for nm, d_ap, lst in (("wq", wq_d, wq_hf), ("wk", wk_d, wk_hf),
                                  ("wv", wv_d, wv_hf)):
                wf = const.tile([128, 256], F32, name=f"{nm}_f{t}")
                nc.sync.dma_start(wf, d_ap[t * 128:(t + 1) * 128, :])
                wb = const.tile([128, 256], F16, name=f"{nm}_h{t}")
                nc.vector.tensor_copy(wb, wf)
                lst.append(wb)
        wo_f = const.tile([128, 1024], F32, name="wo_f")
        nc.scalar.dma_start(wo_f, wo_d)
        wo_hf = const.tile([128, 1024], F16, name="wo_hf")
        nc.vector.tensor_copy(wo_hf, wo_f)
        bq_sb = const.tile([128, 2], F32, name="bq_sb")
        nc.sync.dma_start(bq_sb, bq_d)
        bk_sb = const.tile([128, 2], F32, name="bk_sb")
        nc.sync.dma_start(bk_sb, bk_d)
        bo_f = const.tile([1, HID], F32, name="bo_f")
        nc.scalar.dma_start(bo_f, bo_d)
        bo_hf = const.tile([1, HID], F16, name="bo_hf")
        nc.vector.tensor_copy(bo_hf, bo_f)

        identity = const.tile([128, 128], F32, name="identity")
        make_identity(nc, identity)
        ident_h = const.tile([128, 128], F16, name="ident_h")
        nc.vector.tensor_copy(ident_h, identity)
        ones_hf = const.tile([1, 128], F16, name="ones_hf")
        nc.vector.memset(ones_hf, 1.0)
        # K=2 broadcast selector at partitions 32/33 (value 1/2048 = 2^-11)
        sel2 = const.tile([128, 128], F16, name="sel2")
        nc.sync.dma_start(sel2[32:34, :], sel_d)

        # ---------------- persistent buffers ----------------
        qT_sb = [persist.tile([128, SQ], F16, name=f"qT_sb{g}") for g in range(2)]
        kT_ch = [[persist.tile([128, 512], F16, name=f"kT{g}_{c}")
                  for c in range(skc // 512)] for g in range(2)]
        # augmented V: per head 64 cols = [32 v | mask | 31 zeros]
        vaug = [persist.tile([128, 512], F16, name=f"vaug{s}")
                for s in range(nkc)]
        wtn_all = [persist.tile([128, 512], F16, name=f"wtn{i}")
                   for i in range(4)]
        # vaug init from host (mask col + zeros; v cols overwritten by proj)
        for s in range(nkc):
            nc.scalar.dma_start(vaug[s], vg_d[:, s * 512:(s + 1) * 512])

        # ---------------- phase A helpers ----------------
        def transpose_half(xins, col0):
            """4 tiles -> fp16 [128, 512] chunk of rows col0..col0+128."""
            tp = tpsum.tile([128, 512], F16, tag="tp", name="tp")
            for j in range(4):
                nc.tensor.transpose(
                    tp[:, j * 128:(j + 1) * 128],
                    xins[j][:, col0:col0 + 128], ident_h)
            ch = xt_pool.tile([128, 512], F16, tag="xch", name="xch")
            nc.vector.tensor_copy(ch, tp)
            return ch

        def project_chunk(chunks, w_hf, b_sb, outs):
            for g in range(2):
                ps = ppsum.tile([128, 512], F32, tag="proj", name="ps")
                for t in range(2):
                    nc.tensor.matmul(
                        ps, w_hf[t][:, g * 128:(g + 1) * 128], chunks[t],
                        start=(t == 0), stop=(t == 1))
                nc.scalar.activation(outs[g], ps, AF.Identity,
                                     bias=b_sb[:, g:g + 1], scale=1.0)

        def value_tile(s, xkv):
            vtp = tpsum.tile([128, 256], F16, tag="tp", name="vtp")
            for t in range(2):
                nc.tensor.transpose(
                    vtp[:, t * 128:(t + 1) * 128],
                    xkv[:, 256 + t * 128:256 + (t + 1) * 128], ident_h)
            vT = vt_pool.tile([128, 256], F16, tag="vT", name="vT")
            if s % 2 == 0:
                nc.scalar.activation(vT, vtp, AF.Copy)
            else:
                nc.vector.tensor_copy(vT, vtp)
            vps = ppsum.tile([128, 256], F32, tag="proj", name="vps")
            for t in range(2):
                nc.tensor.matmul(
                    vps, vT[:, t * 128:(t + 1) * 128], wv_hf[t],
                    start=(t == 0), stop=(t == 1))
            dst = vaug[s].rearrange("p (h e) -> p h e", e=64)[:, :, 0:DH]
            src = vps.rearrange("p (h e) -> p h e", e=DH)
            if s % 2 == 0:
                nc.vector.tensor_copy(dst, src)
            else:
                nc.scalar.activation(dst, src, AF.Copy)

        # ---------------- phase A emission ----------------
        def emit_kv_chunk(cch):
            xins = [xkv_tiles[cch * 4 + j] for j in range(4)]
            chunks = [transpose_half(xins, t * 128) for t in range(2)]
            project_chunk(chunks, wk_hf, bk_sb,
                          [kT_ch[g][cch] for g in range(2)])
            for j in range(4):
                value_tile(cch * 4 + j, xins[j])

        nch = skc // 512
        for s in range(8, nkc):
            gather_kv(s)
        for sg in range(SQ // 512):
            xq = xq_all[sg * 4:sg * 4 + 4]
            chunks = [transpose_half(xq, t * 128) for t in range(2)]
            project_chunk(chunks, wq_hf, bq_sb,
                          [qT_sb[g][:, sg * 512:(sg + 1) * 512]
                           for g in range(2)])
        for cch in range(nch):
            emit_kv_chunk(cch)

        # close phase-A PSUM pools, open attention pools over their banks
        phaseA_stack.close()
        st_pool = top.enter_context(
            tc.tile_pool(name="stp", bufs=3, space="PSUM"))
        wt_pool = top.enter_context(
            tc.tile_pool(name="wtp", bufs=2, space="PSUM"))

        # ---------------- phase B: attention ----------------
        def make_tail(qc, wcops, dpack):
            def tail():
                # reciprocal: magic + 2 Newton iterations, output 2048/d fp16
                r0i = dn_pool.tile([8, 512], I32, tag="dp", name="r0i")
                nc.vector.tensor_scalar(r0i, dpack.bitcast(I32), -1, MAGIC,
                                        op0=ALU.mult, op1=ALU.add)
                r0 = r0i.bitcast(F32)
                t1 = dn_pool.tile([8, 512], F32, tag="dp", name="t1")
                nc.vector.tensor_tensor(t1, dpack, r0, op=ALU.mult)
                t1b = dn_pool.tile([8, 512], F32, tag="dp", name="t1b")
                nc.vector.tensor_scalar(t1b, t1, -1.0, 2.0,
                                        op0=ALU.mult, op1=ALU.add)
                r1 = dn_pool.tile([8, 512], F32, tag="dp", name="r1")
                nc.vector.tensor_tensor(r1, r0, t1b, op=ALU.mult)
                t2 = dn_pool.tile([8, 512], F32, tag="dp", name="t2")
                nc.vector.tensor_tensor(t2, dpack, r1, op=ALU.mult)
                t2b = dn_pool.tile([8, 512], F32, tag="dp", name="t2b")
                nc.vector.tensor_scalar(t2b, t2, -RS, 2.0 * RS,
                                        op0=ALU.mult, op1=ALU.add)
                r2h = dn_pool.tile([8, 512], F16, tag="dp", name="r2h")
                nc.vector.tensor_tensor(r2h, r1, t2b, op=ALU.mult)

                # broadcast + normalize
                for g in range(2):
                    for jj in range(2):
                        p = 2 * g + jj
                        rsp = rs_pool.tile([34, 512], F16, tag="rsp",
                                           name="rsp")
                        r = 4 * g + 2 * jj
                        nc.sync.dma_start(rsp[32:33, :], r2h[r:r + 1, :])
                        nc.sync.dma_start(rsp[33:34, :], r2h[r + 1:r + 2, :])
                        bc = wt_pool.tile([128, 512], F32, tag="wt", name="bc")
                        nc.tensor.matmul(bc, sel2[32:34, :], rsp[32:34, :],
                                         start=True, stop=True,
                                         tile_position=(32, 0))
                        nc.vector.tensor_tensor(wtn_all[p], wcops[p], bc,
                                                op=ALU.mult)

                # output projection
                for m in range(4):
                    opsb = wt_pool.tile([128, 512], F32, tag="wt", name="ops")
                    ops = opsb[:, 0:256]
                    for p in range(4):
                        nc.tensor.matmul(
                            ops, wtn_all[p][:, m * 128:(m + 1) * 128],
                            wo_hf[:, p * 256:(p + 1) * 256],
                            start=(p == 0), stop=False,
                            skip_group_check=True)
                    nc.tensor.matmul(ops, ones_hf[0:1, :], bo_hf,
                                     start=False, stop=True,
                                     skip_group_check=True)
                    ob = osb_pool.tile([128, 256], F32, tag="ob", name="ob")
                    nc.scalar.activation(ob, ops, AF.Copy)
                    nc.sync.dma_start(
                        out_d[qc * 512 + m * 128:qc * 512 + (m + 1) * 128, :],
                        ob)
            return tail

        pending_tail = None
        for qc in range(SQ // 512):
            wcops = []
            dpack = dn_pool.tile([8, 512], F32, tag="dp", name="dpack")
            for g in range(2):
                wts = [wt_pool.tile([128, 512], F32, tag="wt", name=f"wt{jj}")
                       for jj in range(2)]
                for kt in range(nkc):
                    for jj in range(2):
                        st = st_pool.tile([128, 1024], F32, tag="st",
                                          name="st")
                        for j2 in range(2):
                            j = 2 * jj + j2
                            nc.tensor.matmul(
                                st[:, j2 * 512:(j2 + 1) * 512],
                                kT_ch[g][kt // 4][32 * j:32 * j + 32,
                                                  (kt % 4) * 128:
                                                  (kt % 4) * 128 + 128],
                                qT_sb[g][32 * j:32 * j + 32,
                                         qc * 512:(qc + 1) * 512],
                                start=True, stop=True,
                                tile_position=(32 * j, 0))
                        if _use_dve(qc, g, jj):
                            p1 = pt_pool.tile([128, 1024], I16, tag="pt",
                                              name="p1")
                            nc.vector.tensor_scalar(
                                p1, st, ASC, B16,
                                op0=ALU.mult, op1=ALU.add)
                            p2 = pt_pool.tile([128, 1024], I16, tag="pt",
                                              name="p2")
                            nc.gpsimd.tensor_scalar(
                                p2, p1, 1, 512,
                                op0=ALU.mult, op1=ALU.add)
                            streams = [p1.bitcast(F16), p2.bitcast(F16)]
                        else:
                            pt = pt_pool.tile([128, 1024], F16, tag="pt",
                                              name="pt")
                            nc.scalar.activation(pt, st, AF.Exp,
                                                 scale=SCALE)
                            streams = [pt]
                        nstr = len(streams)
                        for si, pstr in enumerate(streams):
                            for j2 in range(2):
                                h = 4 * g + 2 * jj + j2
                                nc.tensor.matmul(
                                    wts[jj][64 * j2:64 * j2 + 64, :],
                                    vaug[kt][:, 64 * h:64 * h + 64],
                                    pstr[:, j2 * 512:(j2 + 1) * 512],
                                    start=(kt == 0 and si == 0),
                                    stop=(kt == nkc - 1 and si == nstr - 1),
                                    tile_position=(0, 64 * j2),
                                    skip_group_check=True)



                # evict (frees PSUM), stash denominator rows
                for jj in range(2):
                    wcop = wc_pool.tile([128, 512], F32, tag="wcop",
                                        name="wcop")
                    nc.scalar.activation(wcop, wts[jj], AF.Copy)
                    r = 4 * g + 2 * jj
                    nc.sync.dma_start(dpack[r:r + 1, :], wcop[32:33, :])
                    nc.sync.dma_start(dpack[r + 1:r + 2, :], wcop[96:97, :])
                    wcops.append(wcop)

                # overlap previous qc's normalize+outproj under this qc
                if pending_tail is not None and g == 0:
                    pending_tail()
                    pending_tail = None

            pending_tail = make_tail(qc, wcops, dpack)
        pending_tail()

    nc.finalize()
    return nc


def _get_nc(nkc):
    key = ("nc", nkc)
    if key not in _CACHE:
        _CACHE[key] = _build_nc(nkc)
    return _CACHE[key]


def kernel(query, key, value, mask, Wq, bq, Wk, bk, Wv, bv, Wo, bo,
           _trace=False):
    query = np.asarray(query, np.float32)
    key = np.asarray(key, np.float32)
    value = np.asarray(value, np.float32)
    mask = np.asarray(mask, np.int32)
    Wq = np.ascontiguousarray(np.asarray(Wq, np.float32))
    Wk = np.ascontiguousarray(np.asarray(Wk, np.float32))
    Wv = np.ascontiguousarray(np.asarray(Wv, np.float32))
    Wo = np.ascontiguousarray(np.asarray(Wo, np.float32))
    bq = np.asarray(bq, np.float32)
    bk = np.asarray(bk, np.float32)
    bv = np.asarray(bv, np.float32)
    bo = np.asarray(bo, np.float32)

    # mask compaction: indices of surviving keys per batch, padded to a
    # multiple of 512 with pointers at the zero rows
    idxs = [np.nonzero(mask[b, 0])[0].astype(np.int32) for b in range(2)]
    nk_max = max(len(ix) for ix in idxs)
    nk_max = max(nk_max, 1)
    skc = ((nk_max + 511) // 512) * 512
    nkc = skc // 128

    nc = _get_nc(nkc)

    wo_arr = np.zeros((128, 4, 256), np.float32)
    for p in range(4):
        wo_arr[0:32, p] = Wo[64 * p:64 * p + 32]
        wo_arr[64:96, p] = Wo[64 * p + 32:64 * p + 64]
    wo_arr = np.ascontiguousarray(wo_arr.reshape(128, 1024))
    bq2 = np.ascontiguousarray(bq.reshape(2, 128).T)
    bk2 = np.ascontiguousarray(bk.reshape(2, 128).T)
    bo2 = np.ascontiguousarray((bv @ Wo + bo).reshape(1, 256))
    sel2 = np.zeros((2, 128), np.float16)
    sel2[0, 0:32] = 1.0 / RS
    sel2[1, 64:96] = 1.0 / RS

    # per-batch KV concat with zero pad rows (fp16 on host)
    kv_full = []
    for b in range(2):
        kv = np.zeros((SKP, 2 * HID), np.float16)
        kv[:SK, :HID] = key[b].astype(np.float16)
        kv[:SK, HID:] = value[b].astype(np.float16)
        kv_full.append(kv)

    in_maps = []
    for c in range(8):
        b, qi = divmod(c, 4)
        ix = idxs[b]
        nk = len(ix)
        ix_pad = np.concatenate(
            [ix, np.full(skc - nk, SK, np.int32)])
        mcomp = (np.arange(skc) < nk).astype(np.float16)
        ib = np.ascontiguousarray(ix_pad.reshape(nkc, 128).T)
        # vaug init pattern: per tile, per head: [32 zeros | mask | 31 zeros]
        mb = np.ascontiguousarray(mcomp.reshape(nkc, 128).T)  # [128, nkc]
        vgi = np.zeros((128, nkc, 8, 64), np.float16)
        vgi[:, :, :, 32] = mb[:, :, None]
        vgi = np.ascontiguousarray(vgi.reshape(128, nkc * 512))
        in_maps.append({
            "q_in": np.ascontiguousarray(
                query[b, qi * SQ:(qi + 1) * SQ].astype(np.float16)),
            "kv_in": kv_full[b],
            "vginit": vgi,
            "idx_in": ib,
            "wq": Wq, "wk": Wk, "wv": Wv, "wo_arr": wo_arr,
            "bq2": bq2, "bk2": bk2, "bo2": bo2, "sel2": sel2,
        })

    res = run_bass_kernel_spmd(nc, in_maps, core_ids=list(range(8)),
                               trace=_trace)
    if _trace:
        _CACHE["last_result"] = res

    out = np.empty((2, 4096, HID), np.float32)
    for c in range(8):
        b, qi = divmod(c, 4)
        out[b, qi * SQ:(qi + 1) * SQ] = res.results[c]["out"]
    return out


# revision 41
# speedup vs baseline: 1.0923x; 1.0266x over previous
"""Trainium2 Bass kernel for nn_AttentionBlock (B=2, S=4096, HID=256, 8 heads).

Sharding: 8 cores = 2 batches x 4 query-chunks of 1024 queries.
Each core redundantly computes full K/V projections for its batch (over the
mask-compacted key set), then attention for its 1024 queries over all 8
heads, then the output projection. Host gathers by concatenation.

Key structure (v2):
- Mask compaction: surviving key indices gathered via indirect DMA from a
  host-concatenated [K|V] tensor with appended zero rows; padding slots
  point at the zero rows, and the ones-column of the augmented V carries the
  mask, so masking is exact with no exp bias anywhere.
- Scores computed transposed (sT[k,q]) via 4-way row-packed K=32 fp16
  matmuls, 512 queries per matmul.
- Softmax exp split across engines per head-pair: half the head-pairs use
  the Scalar engine's LUT exp; the other half use a two-sample Schraudolph
  bit-trick exp on the Vector engine (t = A*x + B rounded to int16,
  bitcast to fp16 = 2^(t/1024) approx; summing the B and B+512 samples
  cancels the sawtooth to ~0.5% ripple; the per-softmax constant factor
  cancels in normalization). The extra sample is accumulated by the PE via
  a second PV matmul wave into the same PSUM accumulators.
- Augmented V tiles [128, 512]: per head 64 cols = [32 v | mask | 31 zeros],
  so M=64 PV matmuls fully cover the PSUM rows (no garbage rows) and the
  denominators accumulate alongside.
- Normalization: denominator rows DMA-packed into one [8,512] tile per qc,
  reciprocal via int32 magic-constant + 2 Newton iterations on DVE (scaled
  by 2048 to keep fp16 r values normal), scattered to partitions 32/33 of a
  small tile, broadcast to all 128 partitions with one K=2 matmul against a
  1/2048-valued selector, then one full-tile multiply produces the fp16
  normalized weights for the output projection.
- Output projection: fused K=128 matmuls against zero-padded Wo rows, bias
  (incl. folded bv@Wo) via a K=1 ones matmul.
"""

import numpy as np

import concourse.bacc as bacc
import concourse.bass as bass
from concourse import mybir
from concourse.tile import TileContext
from concourse.masks import make_identity
from concourse.bass_utils import run_bass_kernel_spmd

F32 = mybir.dt.float32
F16 = mybir.dt.float16
I16 = mybir.dt.int16
I32 = mybir.dt.int32
AF = mybir.ActivationFunctionType
ALU = mybir.AluOpType

HID = 256
HEADS = 8
DH = 32
SK = 4096
SKP = 4104  # K/V rows incl. zero-pad rows
SQ = 1024   # queries per core
SCALE = 1.0 / np.sqrt(32.0)
A16 = 1024.0 / np.log(2.0)          # Schraudolph slope (fp16 format)
ASC = A16 * SCALE                   # folded score scale
B16 = 15360.0                       # Schraudolph offset (15*1024)
MAGIC = 0x7EF311C3                  # fp32 reciprocal magic
RS = 2048.0                         # reciprocal output scaling (keeps fp16 normal)

_CACHE = {}

# exp-engine map: (qc, g, jj) -> True if DVE (Schraudolph), False if ACT
def _use_dve(qc, g, jj):
    return jj == 1


def _build_nc(nkc):
    """nkc = number of 128-key tiles after mask compaction (multiple of 4)."""
    skc = nkc * 128
    nc = bacc.Bacc("TRN2", target_bir_lowering=False, debug=False, num_devices=8)

    q_d = nc.dram_tensor("q_in", [SQ, HID], F16, kind="ExternalInput").ap()
    kvh_d = nc.dram_tensor("kv_in", [SKP, 2 * HID], F16, kind="ExternalInput").ap()
    vg_d = nc.dram_tensor("vginit", [128, nkc * 512], F16, kind="ExternalInput").ap()
    i_d = nc.dram_tensor("idx_in", [128, nkc], I32, kind="ExternalInput").ap()
    wq_d = nc.dram_tensor("wq", [HID, HID], F32, kind="ExternalInput").ap()
    wk_d = nc.dram_tensor("wk", [HID, HID], F32, kind="ExternalInput").ap()
    wv_d = nc.dram_tensor("wv", [HID, HID], F32, kind="ExternalInput").ap()
    wo_d = nc.dram_tensor("wo_arr", [128, 1024], F32, kind="ExternalInput").ap()
    bq_d = nc.dram_tensor("bq2", [128, 2], F32, kind="ExternalInput").ap()
    bk_d = nc.dram_tensor("bk2", [128, 2], F32, kind="ExternalInput").ap()
    bo_d = nc.dram_tensor("bo2", [1, HID], F32, kind="ExternalInput").ap()
    sel_d = nc.dram_tensor("sel2", [2, 128], F16, kind="ExternalInput").ap()
    out_d = nc.dram_tensor("out", [SQ, HID], F32, kind="ExternalOutput").ap()

    from contextlib import ExitStack

    with TileContext(nc) as tc, ExitStack() as top:
        const = top.enter_context(tc.tile_pool(name="const", bufs=1))
        persist = top.enter_context(tc.tile_pool(name="persist", bufs=1))
        io_pool = top.enter_context(tc.tile_pool(name="io", bufs=16))
        xt_pool = top.enter_context(tc.tile_pool(name="xt", bufs=3))
        vt_pool = top.enter_context(tc.tile_pool(name="vt", bufs=3))
        pt_pool = top.enter_context(tc.tile_pool(name="pt", bufs=6))
        wc_pool = top.enter_context(tc.tile_pool(name="wc", bufs=8))
        dn_pool = top.enter_context(tc.tile_pool(name="dn", bufs=18))
        rs_pool = top.enter_context(tc.tile_pool(name="rs", bufs=8))
        osb_pool = top.enter_context(tc.tile_pool(name="osb", bufs=4))

        # phase-A PSUM pools are scoped: closed before attention so their
        # banks go to a third st buffer (deeper score->exp pipeline)
        phaseA_stack = ExitStack()
        tpsum = phaseA_stack.enter_context(
            tc.tile_pool(name="tpsum", bufs=4, space="PSUM"))
        ppsum = phaseA_stack.enter_context(
            tc.tile_pool(name="ppsum", bufs=3, space="PSUM"))

        # ---------------- early IO: idx, gathers, q loads ----------------
        idx_sb = const.tile([128, nkc], I32, name="idx_sb")
        nc.sync.dma_start(idx_sb, i_d)

        xkv_tiles = {}

        def gather_kv(s):
            xkv = io_pool.tile([128, 512], F16, tag="xkv", name="xkv")
            nc.gpsimd.indirect_dma_start(
                out=xkv, out_offset=None, in_=kvh_d,
                in_offset=bass.IndirectOffsetOnAxis(
                    ap=idx_sb[:, s:s + 1], axis=0))
            xkv_tiles[s] = xkv

        for s in range(min(8, nkc)):
            gather_kv(s)

        xq_all = []
        for s in range(8):
            xq = io_pool.tile([128, 256], F16, tag="xq", name="xq")
            nc.sync.dma_start(xq, q_d[s * 128:(s + 1) * 128, :])
            xq_all.append(xq)

        # ---------------- constants ----------------
        wq_hf = []
        wk_hf = []
        wv_hf = []
        for t in range(2):
            for nm, d_ap, lst in (("wq", wq_d, wq_hf), ("wk", wk_d, wk_hf),
                                  ("wv", wv_d, wv_hf)):
                wf = const.tile([128, 256], F32, name=f"{nm}_f{t}")
                nc.sync.dma_start(wf, d_ap[t * 128:(t + 1) * 128, :])
                wb = const.tile([128, 256], F16, name=f"{nm}_h{t}")
                nc.vector.tensor_copy(wb, wf)
                lst.append(wb)
        wo_f = const.tile([128, 1024], F32, name="wo_f")
        nc.scalar.dma_start(wo_f, wo_d)
        wo_hf = const.tile([128, 1024], F16, name="wo_hf")
        nc.vector.tensor_copy(wo_hf, wo_f)
        bq_sb = const.tile([128, 2], F32, name="bq_sb")
        nc.sync.dma_start(bq_sb, bq_d)
        bk_sb = const.tile([128, 2], F32, name="bk_sb")
        nc.sync.dma_start(bk_sb, bk_d)
        bo_f = const.tile([1, HID], F32, name="bo_f")
        nc.scalar.dma_start(bo_f, bo_d)
        bo_hf = const.tile([1, HID], F16, name="bo_hf")
        nc.vector.tensor_copy(bo_hf, bo_f)

        identity = const.tile([128, 128], F32, name="identity")
        make_identity(nc, identity)
        ident_h = const.tile([128, 128], F16, name="ident_h")
        nc.vector.tensor_copy(ident_h, identity)
        ones_hf = const.tile([1, 128], F16, name="ones_hf")
        nc.vector.memset(ones_hf, 1.0)
        # K=2 broadcast selector at partitions 32/33 (value 1/2048 = 2^-11)
        sel2 = const.tile([128, 128], F16, name="sel2")
        nc.sync.dma_start(sel2[32:34, :], sel_d)

        # ---------------- persistent buffers ----------------
        qT_sb = [persist.tile([128, SQ], F16, name=f"qT_sb{g}") for g in range(2)]
        kT_ch = [[persist.tile([128, 512], F16, name=f"kT{g}_{c}")
                  for c in range(skc // 512)] for g in range(2)]
        # augmented V: per head 64 cols = [32 v | mask | 31 zeros]
        vaug = [persist.tile([128, 512], F16, name=f"vaug{s}")
                for s in range(nkc)]
        wtn_all = [persist.tile([128, 512], F16, name=f"wtn{i}")
                   for i in range(4)]
        # vaug init from host (mask col + zeros; v cols overwritten by proj)
        for s in range(nkc):
            nc.scalar.dma_start(vaug[s], vg_d[:, s * 512:(s + 1) * 512])

        # ---------------- phase A helpers ----------------
        def transpose_half(xins, col0):
            """4 tiles -> fp16 [128, 512] chunk of rows col0..col0+128."""
            tp = tpsum.tile([128, 512], F16, tag="tp", name="tp")
            for j in range(4):
                nc.tensor.transpose(
                    tp[:, j * 128:(j + 1) * 128],
                    xins[j][:, col0:col0 + 128], ident_h)
            ch = xt_pool.tile([128, 512], F16, tag="xch", name="xch")
            nc.vector.tensor_copy(ch, tp)
            return ch

        def project_chunk(chunks, w_hf, b_sb, outs):
            for g in range(2):
                ps = ppsum.tile([128, 512], F32, tag="proj", name="ps")
                for t in range(2):
                    nc.tensor.matmul(
                        ps, w_hf[t][:, g * 128:(g + 1) * 128], chunks[t],
                        start=(t == 0), stop=(t == 1))
                nc.scalar.activation(outs[g], ps, AF.Identity,
                                     bias=b_sb[:, g:g + 1], scale=1.0)

        def value_tile(s, xkv):
            vtp = tpsum.tile([128, 256], F16, tag="tp", name="vtp")
            for t in range(2):
                nc.tensor.transpose(
                    vtp[:, t * 128:(t + 1) * 128],
                    xkv[:, 256 + t * 128:256 + (t + 1) * 128], ident_h)
            vT = vt_pool.tile([128, 256], F16, tag="vT", name="vT")
            if s % 2 == 0:
                nc.scalar.activation(vT, vtp, AF.Copy)
            else:
                nc.vector.tensor_copy(vT, vtp)
            vps = ppsum.tile([128, 256], F32, tag="proj", name="vps")
            for t in range(2):
                nc.tensor.matmul(
                    vps, vT[:, t * 128:(t + 1) * 128], wv_hf[t],
                    start=(t == 0), stop=(t == 1))
            dst = vaug[s].rearrange("p (h e) -> p h e", e=64)[:, :, 0:DH]
            src = vps.rearrange("p (h e) -> p h e", e=DH)
            if s % 2 == 0:
                nc.vector.tensor_copy(dst, src)
            else:
                nc.scalar.activation(dst, src, AF.Copy)

        # ---------------- phase A emission ----------------
        def emit_kv_chunk(cch):
            xins = [xkv_tiles[cch * 4 + j] for j in range(4)]
            chunks = [transpose_half(xins, t * 128) for t in range(2)]
            project_chunk(chunks, wk_hf, bk_sb,
                          [kT_ch[g][cch] for g in range(2)])
            for j in range(4):
                value_tile(cch * 4 + j, xins[j])

        nch = skc // 512
        for s in range(8, nkc):
            gather_kv(s)
        for sg in range(SQ // 512):
            xq = xq_all[sg * 4:sg * 4 + 4]
            chunks = [transpose_half(xq, t * 128) for t in range(2)]
            project_chunk(chunks, wq_hf, bq_sb,
                          [qT_sb[g][:, sg * 512:(sg + 1) * 512]
                           for g in range(2)])
        for cch in range(nch):
            emit_kv_chunk(cch)

        # close phase-A PSUM pools, open attention pools over their banks
        phaseA_stack.close()
        st_pool = top.enter_context(
            tc.tile_pool(name="stp", bufs=3, space="PSUM"))
        wt_pool = top.enter_context(
            tc.tile_pool(name="wtp", bufs=2, space="PSUM"))

        # ---------------- phase B: attention ----------------
        def make_tail(qc, wcops, dpack):
            def tail():
                # reciprocal: magic + 2 Newton iterations, output 2048/d fp16
                r0i = dn_pool.tile([8, 512], I32, tag="dp", name="r0i")
                nc.vector.tensor_scalar(r0i, dpack.bitcast(I32), -1, MAGIC,
                                        op0=ALU.mult, op1=ALU.add)
                r0 = r0i.bitcast(F32)
                t1 = dn_pool.tile([8, 512], F32, tag="dp", name="t1")
                nc.vector.tensor_tensor(t1, dpack, r0, op=ALU.mult)
                t1b = dn_pool.tile([8, 512], F32, tag="dp", name="t1b")
                nc.vector.tensor_scalar(t1b, t1, -1.0, 2.0,
                                        op0=ALU.mult, op1=ALU.add)
                r1 = dn_pool.tile([8, 512], F32, tag="dp", name="r1")
                nc.vector.tensor_tensor(r1, r0, t1b, op=ALU.mult)
                t2 = dn_pool.tile([8, 512], F32, tag="dp", name="t2")
                nc.vector.tensor_tensor(t2, dpack, r1, op=ALU.mult)
                t2b = dn_pool.tile([8, 512], F32, tag="dp", name="t2b")
                nc.vector.tensor_scalar(t2b, t2, -RS, 2.0 * RS,
                                        op0=ALU.mult, op1=ALU.add)
                r2h = dn_pool.tile([8, 512], F16, tag="dp", name="r2h")
                nc.vector.tensor_tensor(r2h, r1, t2b, op=ALU.mult)

                # broadcast + normalize
                for g in range(2):
                    for jj in range(2):
                        p = 2 * g + jj
                        rsp = rs_pool.tile([34, 512], F16, tag="rsp",
                                           name="rsp")
                        r = 4 * g + 2 * jj
                        nc.sync.dma_start(rsp[32:33, :], r2h[r:r + 1, :])
                        nc.sync.dma_start(rsp[33:34, :], r2h[r + 1:r + 2, :])
                        bc = wt_pool.tile([128, 512], F32, tag="wt", name="bc")
                        nc.tensor.matmul(bc, sel2[32:34, :], rsp[32:34, :],
                                         start=True, stop=True,
                                         tile_position=(32, 0))
                        nc.vector.tensor_tensor(wtn_all[p], wcops[p], bc,
                                                op=ALU.mult)

                # output projection
                for m in range(4):
                    opsb = wt_pool.tile([128, 512], F32, tag="wt", name="ops")
                    ops = opsb[:, 0:256]
                    for p in range(4):
                        nc.tensor.matmul(
                            ops, wtn_all[p][:, m * 128:(m + 1) * 128],
                            wo_hf[:, p * 256:(p + 1) * 256],
                            start=(p == 0), stop=False,
                            skip_group_check=True)
                    nc.tensor.matmul(ops, ones_hf[0:1, :], bo_hf,
                                     start=False, stop=True,
                                     skip_group_check=True)
                    ob = osb_pool.tile([128, 256], F32, tag="ob", name="ob")
                    nc.scalar.activation(ob, ops, AF.Copy)
                    nc.sync.dma_start(
                        out_d[qc * 512 + m * 128:qc * 512 + (m + 1) * 128, :],
                        ob)
            return tail

        pending_tail = None
        for qc in range(SQ // 512):
            wcops = []
            dpack = dn_pool.tile([8, 512], F32, tag="dp", name="dpack")
            for g in range(2):
                wts = [wt_pool.tile([128, 512], F32, tag="wt", name=f"wt{jj}")
                       for jj in range(2)]
                for kt in range(nkc):
                    for jj in range(2):
                        st = st_pool.tile([128, 1024], F32, tag="st",
                                          name="st")
                        for j2 in range(2):
                            j = 2 * jj + j2
                            nc.tensor.matmul(
                                st[:, j2 * 512:(j2 + 1) * 512],
                                kT_ch[g][kt // 4][32 * j:32 * j + 32,
                                                  (kt % 4) * 128:
                                                  (kt % 4) * 128 + 128],
                                qT_sb[g][32 * j:32 * j + 32,
                                         qc * 512:(qc + 1) * 512],
                                start=True, stop=True,
                                tile_position=(32 * j, 0))
                        if _use_dve(qc, g, jj):
                            p1 = pt_pool.tile([128, 1024], I16, tag="pt",
                                              name="p1")
                            nc.vector.tensor_scalar(
                                p1, st, ASC, B16,
                                op0=ALU.mult, op1=ALU.add)
                            p2 = pt_pool.tile([128, 1024], I16, tag="pt",
                                              name="p2")
                            nc.gpsimd.tensor_scalar(
                                p2, p1, 1, 512,
                                op0=ALU.mult, op1=ALU.add)
                            streams = [p1.bitcast(F16), p2.bitcast(F16)]
                        else:
                            pt = pt_pool.tile([128, 1024], F16, tag="pt",
                                              name="pt")
                            nc.scalar.activation(pt, st, AF.Exp,
                                                 scale=SCALE)
                            streams = [pt]
                        nstr = len(streams)
                        for si, pstr in enumerate(streams):
                            for j2 in range(2):
                                h = 4 * g + 2 * jj + j2
                                nc.tensor.matmul(
                                    wts[jj][64 * j2:64 * j2 + 64, :],
                                    vaug[kt][:, 64 * h:64 * h + 64],
                                    pstr[:, j2 * 512:(j2 + 1) * 512],
                                    start=(kt == 0 and si == 0),
                                    stop=(kt == nkc - 1 and si == nstr - 1),
                                    tile_position=(0, 64 * j2),
                                    skip_group_check=True)



                # evict (frees PSUM), stash denominator rows
                for jj in range(2):
                    wcop = wc_pool.tile([128, 512], F32, tag="wcop",
                                        name="wcop")
                    nc.scalar.activation(wcop, wts[jj], AF.Copy)
                    r = 4 * g + 2 * jj
                    nc.sync.dma_start(dpack[r:r + 1, :], wcop[32:33, :])
                    nc.sync.dma_start(dpack[r + 1:r + 2, :], wcop[96:97, :])
                    wcops.append(wcop)

                # overlap previous qc's normalize+outproj under this qc
                if pending_tail is not None and g == 0:
                    pending_tail()
                    pending_tail = None

            pending_tail = make_tail(qc, wcops, dpack)
        pending_tail()

    nc.finalize()
    return nc


def _get_nc(nkc):
    key = ("nc", nkc)
    if key not in _CACHE:
        _CACHE[key] = _build_nc(nkc)
    return _CACHE[key]


def kernel(query, key, value, mask, Wq, bq, Wk, bk, Wv, bv, Wo, bo,
           _trace=False):
    query = np.asarray(query, np.float32)
    key = np.asarray(key, np.float32)
    value = np.asarray(value, np.float32)
    mask = np.asarray(mask, np.int32)
    Wq = np.ascontiguousarray(np.asarray(Wq, np.float32))
    Wk = np.ascontiguousarray(np.asarray(Wk, np.float32))
    Wv = np.ascontiguousarray(np.asarray(Wv, np.float32))
    Wo = np.ascontiguousarray(np.asarray(Wo, np.float32))
    bq = np.asarray(bq, np.float32)
    bk = np.asarray(bk, np.float32)
    bv = np.asarray(bv, np.float32)
    bo = np.asarray(bo, np.float32)

    # mask compaction: indices of surviving keys per batch, padded to a
    # multiple of 512 with pointers at the zero rows
    idxs = [np.nonzero(mask[b, 0])[0].astype(np.int32) for b in range(2)]
    nk_max = max(len(ix) for ix in idxs)
    nk_max = max(nk_max, 1)
    skc = ((nk_max + 511) // 512) * 512
    nkc = skc // 128

    nc = _get_nc(nkc)

    wo_arr = np.zeros((128, 4, 256), np.float32)
    for p in range(4):
        wo_arr[0:32, p] = Wo[64 * p:64 * p + 32]
        wo_arr[64:96, p] = Wo[64 * p + 32:64 * p + 64]
    wo_arr = np.ascontiguousarray(wo_arr.reshape(128, 1024))
    bq2 = np.ascontiguousarray(bq.reshape(2, 128).T)
    bk2 = np.ascontiguousarray(bk.reshape(2, 128).T)
    bo2 = np.ascontiguousarray((bv @ Wo + bo).reshape(1, 256))
    sel2 = np.zeros((2, 128), np.float16)
    sel2[0, 0:32] = 1.0 / RS
    sel2[1, 64:96] = 1.0 / RS

    # per-batch KV concat with zero pad rows (fp16 on host)
    kv_full = []
    for b in range(2):
        kv = np.zeros((SKP, 2 * HID), np.float16)
        kv[:SK, :HID] = key[b].astype(np.float16)
        kv[:SK, HID:] = value[b].astype(np.float16)
        kv_full.append(kv)

    in_maps = []
    for c in range(8):
        b, qi = divmod(c, 4)
        ix = idxs[b]
        nk = len(ix)
        ix_pad = np.concatenate(
            [ix, np.full(skc - nk, SK, np.int32)])
        mcomp = (np.arange(skc) < nk).astype(np.float16)
        ib = np.ascontiguousarray(ix_pad.reshape(nkc, 128).T)
        # vaug init pattern: per tile, per head: [32 zeros | mask | 31 zeros]
        mb = np.ascontiguousarray(mcomp.reshape(nkc, 128).T)  # [128, nkc]
        vgi = np.zeros((128, nkc, 8, 64), np.float16)
        vgi[:, :, :, 32] = mb[:, :, None]
        vgi = np.ascontiguousarray(vgi.reshape(128, nkc * 512))
        in_maps.append({
            "q_in": np.ascontiguousarray(
                query[b, qi * SQ:(qi + 1) * SQ].astype(np.float16)),
            "kv_in": kv_full[b],
            "vginit": vgi,
            "idx_in": ib,
            "wq": Wq, "wk": Wk, "wv": Wv, "wo_arr": wo_arr,
            "bq2": bq2, "bk2": bk2, "bo2": bo2, "sel2": sel2,
        })

    res = run_bass_kernel_spmd(nc, in_maps, core_ids=list(range(8)),
                               trace=_trace)
    if _trace:
        _CACHE["last_result"] = res

    out = np.empty((2, 4096, HID), np.float32)
    for c in range(8):
        b, qi = divmod(c, 4)
        out[b, qi * SQ:(qi + 1) * SQ] = res.results[c]["out"]
    return out
